# revision 4
# baseline (speedup 1.0000x reference)
"""MoE (top-2 of 8 experts, SwiGLU FFN) for 8 Trainium2 NeuronCores.

Strategy: expert parallelism with 2-slot load balancing. The router /
RMSNorm / top-k dispatch are O(T*D) host-side numpy; each NeuronCore runs
the SwiGLU FFN over two token slots (capacities sA >= sB), each slot bound
to one expert's weights. Every expert owns exactly two slots somewhere in
the fleet, so its token list is split across them; slot sizes are chosen
so all 16 slots cover the (data-dependent) per-expert token counts with
minimal padding. All matmuls run in bf16 with fp32 PSUM accumulation,
feature-major layout:

  slot computes   G.T = Wg.T @ Xg.T   [F, C]   (lhsT = Wg, natural layout)
                  U.T = Wu.T @ Xg.T   [F, C]
                  H.T = silu(G.T) * U.T
                  Y.T = Wd.T @ H.T    [D, C]   (lhsT = Wd, natural layout)

The host scales rows of Y by the renormalized top-2 softmax weight and
scatter-adds into the output.
"""

import numpy as np
import ml_dtypes

import concourse.bass as bass  # noqa: F401  (engine handles live on nc)
import concourse.mybir as mybir
import concourse.tile as tile
from concourse import bacc, bass_utils

EPS = 1e-6
TOP_K = 2
N_CORES = 8
P = 128

# Set by a test harness to capture profile info; default path is untouched.
TRACE = False
TRACE_KWARGS = {}
LAST_RESULTS = None

_PROG_CACHE = {}


def _ns_list(NT):
    """Split NT tokens into near-equal matmul streams (<=512 each).

    Near-equal (rather than 512+remainder) keeps every stream long enough
    (>= ~256 cycles) that the per-matmul LDWEIGHTS (~97ns) stays hidden
    behind the previous matmul's stream."""
    pieces = max(1, -(-NT // 512))
    base, rem = divmod(NT, pieces)
    out, o = [], 0
    for i in range(pieces):
        nn = base + (1 if i < rem else 0)
        out.append((o, nn))
        o += nn
    return out


def _build_program(sA, sB, D, F):
    """Two-slot SwiGLU FFN: yt[D, sA+sB] = [ffnA(xgt[:, :sA]), ffnB(xgt[:, sA:])]
    (transposed layouts), with independent weight sets per slot."""
    assert sA % 128 == 0 and sB % 128 == 0
    C = sA + sB
    KD = D // P   # contraction tiles over D (gate/up)
    KF = F // P   # contraction tiles over F (down)
    MF = F // P   # output F tiles (gate/up)
    MD = D // P   # output D tiles (down)
    bf = mybir.dt.bfloat16
    f32 = mybir.dt.float32
    AF = mybir.ActivationFunctionType

    nc = bacc.Bacc("TRN2", target_bir_lowering=False, debug=False)
    xgt = nc.dram_tensor("xgt", [D, C], bf, kind="ExternalInput").ap()
    w_in = {}
    for s in ("a", "b"):
        w_in[s] = (
            nc.dram_tensor(f"wg_{s}", [D, F], bf, kind="ExternalInput").ap(),
            nc.dram_tensor(f"wu_{s}", [D, F], bf, kind="ExternalInput").ap(),
            nc.dram_tensor(f"wd_{s}", [F, D], bf, kind="ExternalInput").ap(),
        )
    yt = nc.dram_tensor("yt", [D, C], f32, kind="ExternalOutput").ap()

    slots = [("a", 0, sA), ("b", sA, sB)]

    with tile.TileContext(nc) as tc:
        with (
            tc.tile_pool(name="xg", bufs=1) as xg_pool,
            tc.tile_pool(name="h", bufs=1) as h_pool,
            tc.tile_pool(name="wgu", bufs=6) as wgu_pool,
            tc.tile_pool(name="wdp", bufs=2) as wd_pool,
            tc.tile_pool(name="sg", bufs=4) as sg_pool,
            tc.tile_pool(name="ot", bufs=4) as o_pool,
            tc.tile_pool(name="ps", bufs=8, space="PSUM") as ps_pool,
        ):
            # whole gathered token block, K(D) on partitions: [128, KD, C].
            # Loaded slot-A-first so the first matmuls wait on less DMA.
            xg_t = xg_pool.tile([P, KD, C], bf)
            for (_, c0, NT) in slots:
                for k in range(KD):
                    nc.sync.dma_start(xg_t[:, k, c0:c0 + NT],
                                      xgt[k * P:(k + 1) * P, c0:c0 + NT])

            for (s, c0, NT) in slots:
                wg, wu, wd = w_in[s]
                ns_list = _ns_list(NT)
                h_t = h_pool.tile([P, MF, max(sA, sB)], bf, tag="h")

                # ---- gate/up projections + silu*mul -> H.T slot ----
                for mg in range(MF // 2):
                    wg_t = wgu_pool.tile([P, KD, 256], bf, tag="wgu")
                    wu_t = wgu_pool.tile([P, KD, 256], bf, tag="wgu")
                    for k in range(KD):
                        nc.sync.dma_start(
                            wg_t[:, k, :],
                            wg[k * P:(k + 1) * P, mg * 256:(mg + 1) * 256])
                        nc.sync.dma_start(
                            wu_t[:, k, :],
                            wu[k * P:(k + 1) * P, mg * 256:(mg + 1) * 256])
                    for mi in range(2):
                        m = mg * 2 + mi
                        for (n0, nn) in ns_list:
                            ps_g = ps_pool.tile([P, 512], f32, tag="ps")
                            ps_u = ps_pool.tile([P, 512], f32, tag="ps")
                            for k in range(KD):
                                nc.tensor.matmul(
                                    ps_g[:, :nn],
                                    wg_t[:, k, mi * P:(mi + 1) * P],
                                    xg_t[:, k, c0 + n0:c0 + n0 + nn],
                                    start=(k == 0), stop=(k == KD - 1))
                            for k in range(KD):
                                nc.tensor.matmul(
                                    ps_u[:, :nn],
                                    wu_t[:, k, mi * P:(mi + 1) * P],
                                    xg_t[:, k, c0 + n0:c0 + n0 + nn],
                                    start=(k == 0), stop=(k == KD - 1))
                            sg_t = sg_pool.tile([P, 512], f32, tag="sg")
                            nc.scalar.activation(sg_t[:, :nn], ps_g[:, :nn], AF.Silu)
                            nc.vector.tensor_mul(
                                h_t[:, m, n0:n0 + nn], sg_t[:, :nn], ps_u[:, :nn])

                # ---- down projection -> Y.T slot ----
                for mgd in range(MD // 2):
                    wd_t = wd_pool.tile([P, KF, 256], bf, tag="wd")
                    for k in range(KF):
                        nc.sync.dma_start(
                            wd_t[:, k, :],
                            wd[k * P:(k + 1) * P, mgd * 256:(mgd + 1) * 256])
                    for mi in range(2):
                        m = mgd * 2 + mi
                        for (n0, nn) in ns_list:
                            ps_d = ps_pool.tile([P, 512], f32, tag="ps")
                            for k in range(KF):
                                nc.tensor.matmul(
                                    ps_d[:, :nn],
                                    wd_t[:, k, mi * P:(mi + 1) * P],
                                    h_t[:, k, n0:n0 + nn],
                                    start=(k == 0), stop=(k == KF - 1))
                            o_t = o_pool.tile([P, 512], f32, tag="ot")
                            nc.vector.tensor_copy(o_t[:, :nn], ps_d[:, :nn])
                            nc.sync.dma_start(
                                yt[m * P:(m + 1) * P, c0 + n0:c0 + n0 + nn],
                                o_t[:, :nn])

    nc.compile()
    return nc


def _get_program(sA, sB, D, F):
    key = (sA, sB, D, F)
    if key not in _PROG_CACHE:
        _PROG_CACHE[key] = _build_program(sA, sB, D, F)
    return _PROG_CACHE[key]


def _assign_slots(counts):
    """Pick slot sizes (sA >= sB) and map each expert to two slots so all 16
    slots (8x sA + 8x sB) cover the per-expert token counts with minimal
    total capacity. Returns (sA, sB, pairs) where pairs[e] in
    {('A','A'), ('A','B'), ('B','B')}."""
    E = len(counts)
    max_c = int(counts.max())
    total = int(counts.sum())
    base = max(256, int(np.ceil(total / E / 128.0)) * 128)

    def greedy(sA, sB):
        order = np.argsort(-counts)
        nA, nB = E, E
        pairs = [None] * E
        for e in order:
            n = int(counts[e])
            for cap, need in ((2 * sB, ("B", "B")), (sA + sB, ("A", "B")),
                              (2 * sA, ("A", "A"))):
                a_need = need.count("A")
                b_need = need.count("B")
                if n <= cap and nA >= a_need and nB >= b_need:
                    pairs[e] = need
                    nA -= a_need
                    nB -= b_need
                    break
            else:
                return None
        return pairs

    for extra in range(0, 16 * 128, 128):
        s_tot = base + extra
        sA0 = int(np.ceil(s_tot / 2 / 128.0)) * 128
        for sA in range(sA0, s_tot + 1, 128):
            sB = s_tot - sA
            if sB <= 0 or 2 * sA < max_c:
                continue
            pairs = greedy(sA, sB)
            if pairs is not None:
                return sA, sB, pairs
    raise RuntimeError("no feasible slot assignment")


def kernel(hidden_states, ln_weight, w_router, w_gate, w_up, w_down):
    global LAST_RESULTS
    hs = np.asarray(hidden_states, dtype=np.float32)
    ln_w = np.asarray(ln_weight, dtype=np.float32)
    w_r = np.asarray(w_router, dtype=np.float32)
    w_gate = np.asarray(w_gate)
    w_up = np.asarray(w_up)
    w_down = np.asarray(w_down)

    B, S, D = hs.shape
    T = B * S
    E, _, F = w_gate.shape
    bf = ml_dtypes.bfloat16

    # ---- host: RMSNorm + router + top-2 dispatch (O(T*D), exact fp32) ----
    x = hs.reshape(T, D)
    var = np.mean(x * x, axis=-1, keepdims=True)
    xn = x * (1.0 / np.sqrt(var + EPS)) * ln_w
    router_logits = xn @ w_r                      # [T, E]
    lm = router_logits.max(-1, keepdims=True)
    probs = np.exp(router_logits - lm)
    probs /= probs.sum(-1, keepdims=True)
    top_idx = np.argpartition(-probs, TOP_K - 1, axis=-1)[:, :TOP_K]  # [T, k]
    top_vals = np.take_along_axis(probs, top_idx, axis=-1)
    top_vals = top_vals / top_vals.sum(-1, keepdims=True)

    flat_expert = top_idx.ravel()
    flat_token = np.repeat(np.arange(T, dtype=np.int64), TOP_K)
    flat_w = top_vals.ravel().astype(np.float32)
    counts = np.bincount(flat_expert, minlength=E)

    # ---- slot assignment: each expert -> 2 slots over the fleet ----
    sA, sB, pairs = _assign_slots(counts)
    free = {"A": list(range(N_CORES)), "B": list(range(N_CORES))}
    cap = {"A": sA, "B": sB}
    # slot_of_core[c] = {"A": (expert, rows, w), "B": ...}
    slot_data = {c: {} for c in range(N_CORES)}
    for e in range(E):
        rows = flat_token[flat_expert == e]
        ws = flat_w[flat_expert == e]
        off = 0
        for kind in pairs[e]:
            c = free[kind].pop()
            take = min(cap[kind], len(rows) - off)
            slot_data[c][kind] = (e, rows[off:off + take], ws[off:off + take])
            off += take
        assert off == len(rows), (e, off, len(rows))

    xnT_b = np.ascontiguousarray(xn.T).astype(bf)  # [D, T]
    wg_b = [np.ascontiguousarray(w_gate[e]).astype(bf) for e in range(E)]
    wu_b = [np.ascontiguousarray(w_up[e]).astype(bf) for e in range(E)]
    wd_b = [np.ascontiguousarray(w_down[e]).astype(bf) for e in range(E)]

    in_maps = []
    scatter = []  # per core: list of (col_offset, rows, w)
    for c in range(N_CORES):
        xgt = np.zeros((D, sA + sB), dtype=bf)
        im = {"xgt": xgt}
        sc = []
        for kind, c0 in (("A", 0), ("B", sA)):
            e, rows, ws = slot_data[c].get(kind, (0, np.empty(0, np.int64),
                                                  np.empty(0, np.float32)))
            xgt[:, c0:c0 + len(rows)] = xnT_b[:, rows]
            sfx = "a" if kind == "A" else "b"
            im[f"wg_{sfx}"] = wg_b[e]
            im[f"wu_{sfx}"] = wu_b[e]
            im[f"wd_{sfx}"] = wd_b[e]
            sc.append((c0, rows, ws))
        in_maps.append(im)
        scatter.append(sc)

    # ---- device: two expert slots per core ----
    nc = _get_program(sA, sB, D, F)
    res = bass_utils.run_bass_kernel_spmd(
        nc, in_maps, core_ids=list(range(N_CORES)),
        trace=TRACE, **TRACE_KWARGS)
    LAST_RESULTS = res

    # ---- host: combine-weight scale + scatter-add ----
    out = np.zeros((T, D), dtype=np.float32)
    for c in range(N_CORES):
        yt = res.results[c]["yt"]
        for (c0, rows, ws) in scatter[c]:
            if len(rows) == 0:
                continue
            y = yt[:, c0:c0 + len(rows)].T          # [n, D] fp32
            out[rows] += ws[:, None] * y
    return out.reshape(B, S, D), router_logits


# revision 7
# speedup vs baseline: 1.1771x; 1.1771x over previous
"""MoE (top-2 of 8 experts, SwiGLU FFN) for 8 Trainium2 NeuronCores.

Strategy: expert parallelism with 2-slot load balancing. The router /
RMSNorm / top-k dispatch are O(T*D) host-side numpy; each NeuronCore runs
the SwiGLU FFN over two token slots (capacities sA >= sB), each slot bound
to one expert's weights. Every expert owns exactly two slots somewhere in
the fleet, so its token list is split across them; slot sizes are chosen
so all 16 slots cover the (data-dependent) per-expert token counts with
minimal padding. All matmuls run in bf16 with fp32 PSUM accumulation,
feature-major layout:

  slot computes   G.T = Wg.T @ Xg.T   [F, C]   (lhsT = Wg, natural layout)
                  U.T = Wu.T @ Xg.T   [F, C]
                  H.T = silu(G.T) * U.T
                  Y.T = Wd.T @ H.T    [D, C]   (lhsT = Wd, natural layout)

The host scales rows of Y by the renormalized top-2 softmax weight and
scatter-adds into the output.
"""

import numpy as np
import ml_dtypes

import concourse.bass as bass  # noqa: F401  (engine handles live on nc)
import concourse.mybir as mybir
import concourse.tile as tile
from concourse import bacc, bass_utils

EPS = 1e-6
TOP_K = 2
N_CORES = 8
P = 128

# Set by a test harness to capture profile info; default path is untouched.
TRACE = False
TRACE_KWARGS = {}
LAST_RESULTS = None

_PROG_CACHE = {}


def _ns_list(NT):
    """Split NT tokens into near-equal matmul streams (<=512 each).

    Near-equal (rather than 512+remainder) keeps every stream long enough
    (>= ~256 cycles) that the per-matmul LDWEIGHTS (~97ns) stays hidden
    behind the previous matmul's stream."""
    pieces = max(1, -(-NT // 512))
    base, rem = divmod(NT, pieces)
    out, o = [], 0
    for i in range(pieces):
        nn = base + (1 if i < rem else 0)
        out.append((o, nn))
        o += nn
    return out


def _build_program(sA, sB, D, F):
    """Two-slot SwiGLU FFN: yt[D, sA+sB] = [ffnA(xgt[:, :sA]), ffnB(xgt[:, sA:])]
    (transposed layouts), with independent weight sets per slot."""
    assert sA % 128 == 0 and sB % 128 == 0
    C = sA + sB
    KD = D // P   # contraction tiles over D (gate/up)
    KF = F // P   # contraction tiles over F (down)
    MF = F // P   # output F tiles (gate/up)
    MD = D // P   # output D tiles (down)
    bf = mybir.dt.bfloat16
    f32 = mybir.dt.float32
    AF = mybir.ActivationFunctionType

    nc = bacc.Bacc("TRN2", target_bir_lowering=False, debug=False)
    xgt = nc.dram_tensor("xgt", [D, C], bf, kind="ExternalInput").ap()
    w_in = {}
    for s in ("a", "b"):
        w_in[s] = (
            nc.dram_tensor(f"wg_{s}", [D, F], bf, kind="ExternalInput").ap(),
            nc.dram_tensor(f"wu_{s}", [D, F], bf, kind="ExternalInput").ap(),
            nc.dram_tensor(f"wd_{s}", [F, D], bf, kind="ExternalInput").ap(),
        )
    yt = nc.dram_tensor("yt", [D, C], f32, kind="ExternalOutput").ap()

    slots = [("a", 0, sA), ("b", sA, sB)]

    with tile.TileContext(nc) as tc:
        with (
            tc.tile_pool(name="xg", bufs=1) as xg_pool,
            tc.tile_pool(name="h", bufs=1) as h_pool,
            tc.tile_pool(name="wgu", bufs=6) as wgu_pool,
            tc.tile_pool(name="wdp", bufs=2) as wd_pool,
            tc.tile_pool(name="sg", bufs=4) as sg_pool,
            tc.tile_pool(name="ot", bufs=4) as o_pool,
            tc.tile_pool(name="ps", bufs=8, space="PSUM") as ps_pool,
        ):
            # whole gathered token block, K(D) on partitions: [128, KD, C].
            # Loaded slot-A-first and per ns-piece so the first matmul group
            # only waits for ~0.7MB of DMA instead of the whole block.
            xg_t = xg_pool.tile([P, KD, C], bf)
            for (_, c0, NT) in slots:
                for (n0, nn) in _ns_list(NT):
                    for k in range(KD):
                        nc.sync.dma_start(
                            xg_t[:, k, c0 + n0:c0 + n0 + nn],
                            xgt[k * P:(k + 1) * P, c0 + n0:c0 + n0 + nn])

            for (s, c0, NT) in slots:
                wg, wu, wd = w_in[s]
                ns_list = _ns_list(NT)
                h_t = h_pool.tile([P, MF, max(sA, sB)], bf, tag="h")

                # ---- gate/up projections + silu*mul -> H.T slot ----
                for mg in range(MF // 2):
                    wg_t = wgu_pool.tile([P, KD, 256], bf, tag="wgu")
                    wu_t = wgu_pool.tile([P, KD, 256], bf, tag="wgu")
                    for k in range(KD):
                        nc.sync.dma_start(
                            wg_t[:, k, :],
                            wg[k * P:(k + 1) * P, mg * 256:(mg + 1) * 256])
                        nc.sync.dma_start(
                            wu_t[:, k, :],
                            wu[k * P:(k + 1) * P, mg * 256:(mg + 1) * 256])
                    for mi in range(2):
                        m = mg * 2 + mi
                        for (n0, nn) in ns_list:
                            ps_g = ps_pool.tile([P, 512], f32, tag="ps")
                            ps_u = ps_pool.tile([P, 512], f32, tag="ps")
                            for k in range(KD):
                                nc.tensor.matmul(
                                    ps_g[:, :nn],
                                    wg_t[:, k, mi * P:(mi + 1) * P],
                                    xg_t[:, k, c0 + n0:c0 + n0 + nn],
                                    start=(k == 0), stop=(k == KD - 1))
                            for k in range(KD):
                                nc.tensor.matmul(
                                    ps_u[:, :nn],
                                    wu_t[:, k, mi * P:(mi + 1) * P],
                                    xg_t[:, k, c0 + n0:c0 + n0 + nn],
                                    start=(k == 0), stop=(k == KD - 1))
                            sg_t = sg_pool.tile([P, 512], f32, tag="sg")
                            nc.scalar.activation(sg_t[:, :nn], ps_g[:, :nn], AF.Silu)
                            nc.vector.tensor_mul(
                                h_t[:, m, n0:n0 + nn], sg_t[:, :nn], ps_u[:, :nn])

                # ---- down projection -> Y.T slot ----
                for mgd in range(MD // 2):
                    wd_t = wd_pool.tile([P, KF, 256], bf, tag="wd")
                    for k in range(KF):
                        nc.sync.dma_start(
                            wd_t[:, k, :],
                            wd[k * P:(k + 1) * P, mgd * 256:(mgd + 1) * 256])
                    for mi in range(2):
                        m = mgd * 2 + mi
                        for (n0, nn) in ns_list:
                            ps_d = ps_pool.tile([P, 512], f32, tag="ps")
                            for k in range(KF):
                                nc.tensor.matmul(
                                    ps_d[:, :nn],
                                    wd_t[:, k, mi * P:(mi + 1) * P],
                                    h_t[:, k, n0:n0 + nn],
                                    start=(k == 0), stop=(k == KF - 1))
                            o_t = o_pool.tile([P, 512], f32, tag="ot")
                            nc.vector.tensor_copy(o_t[:, :nn], ps_d[:, :nn])
                            nc.sync.dma_start(
                                yt[m * P:(m + 1) * P, c0 + n0:c0 + n0 + nn],
                                o_t[:, :nn])

    nc.compile()
    return nc


def _get_program(sA, sB, D, F):
    key = (sA, sB, D, F)
    if key not in _PROG_CACHE:
        _PROG_CACHE[key] = _build_program(sA, sB, D, F)
    return _PROG_CACHE[key]


def _assign_slots(counts):
    """Pick slot sizes (sA >= sB) and map each expert to two slots so all 16
    slots (8x sA + 8x sB) cover the per-expert token counts with minimal
    total capacity. Returns (sA, sB, pairs) where pairs[e] in
    {('A','A'), ('A','B'), ('B','B')}."""
    E = len(counts)
    G = 16  # slot-size granularity (keeps DMA rows 32B-aligned for bf16)
    max_c = int(counts.max())
    total = int(counts.sum())
    base = max(256, int(np.ceil(total / E / G)) * G)

    def greedy(sA, sB):
        order = np.argsort(-counts)
        nA, nB = E, E
        pairs = [None] * E
        for e in order:
            n = int(counts[e])
            for cap, need in ((2 * sB, ("B", "B")), (sA + sB, ("A", "B")),
                              (2 * sA, ("A", "A"))):
                a_need = need.count("A")
                b_need = need.count("B")
                if n <= cap and nA >= a_need and nB >= b_need:
                    pairs[e] = need
                    nA -= a_need
                    nB -= b_need
                    break
            else:
                return None
        return pairs

    for extra in range(0, 256 * G, G):
        s_tot = base + extra
        sA0 = int(np.ceil(s_tot / 2 / G)) * G
        for sA in range(sA0, s_tot + 1, G):
            sB = s_tot - sA
            if sB <= 0 or 2 * sA < max_c:
                continue
            pairs = greedy(sA, sB)
            if pairs is not None:
                return sA, sB, pairs
    raise RuntimeError("no feasible slot assignment")


def kernel(hidden_states, ln_weight, w_router, w_gate, w_up, w_down):
    global LAST_RESULTS
    hs = np.asarray(hidden_states, dtype=np.float32)
    ln_w = np.asarray(ln_weight, dtype=np.float32)
    w_r = np.asarray(w_router, dtype=np.float32)
    w_gate = np.asarray(w_gate)
    w_up = np.asarray(w_up)
    w_down = np.asarray(w_down)

    B, S, D = hs.shape
    T = B * S
    E, _, F = w_gate.shape
    bf = ml_dtypes.bfloat16

    # ---- host: RMSNorm + router + top-2 dispatch (O(T*D), exact fp32) ----
    x = hs.reshape(T, D)
    var = np.mean(x * x, axis=-1, keepdims=True)
    xn = x * (1.0 / np.sqrt(var + EPS)) * ln_w
    router_logits = xn @ w_r                      # [T, E]
    lm = router_logits.max(-1, keepdims=True)
    probs = np.exp(router_logits - lm)
    probs /= probs.sum(-1, keepdims=True)
    top_idx = np.argpartition(-probs, TOP_K - 1, axis=-1)[:, :TOP_K]  # [T, k]
    top_vals = np.take_along_axis(probs, top_idx, axis=-1)
    top_vals = top_vals / top_vals.sum(-1, keepdims=True)

    flat_expert = top_idx.ravel()
    flat_token = np.repeat(np.arange(T, dtype=np.int64), TOP_K)
    flat_w = top_vals.ravel().astype(np.float32)
    counts = np.bincount(flat_expert, minlength=E)

    # ---- slot assignment: each expert -> 2 slots over the fleet ----
    sA, sB, pairs = _assign_slots(counts)
    free = {"A": list(range(N_CORES)), "B": list(range(N_CORES))}
    cap = {"A": sA, "B": sB}
    # slot_of_core[c] = {"A": (expert, rows, w), "B": ...}
    slot_data = {c: {} for c in range(N_CORES)}
    for e in range(E):
        rows = flat_token[flat_expert == e]
        ws = flat_w[flat_expert == e]
        off = 0
        for kind in pairs[e]:
            c = free[kind].pop()
            take = min(cap[kind], len(rows) - off)
            slot_data[c][kind] = (e, rows[off:off + take], ws[off:off + take])
            off += take
        assert off == len(rows), (e, off, len(rows))

    xnT_b = np.ascontiguousarray(xn.T).astype(bf)  # [D, T]
    wg_b = [np.ascontiguousarray(w_gate[e]).astype(bf) for e in range(E)]
    wu_b = [np.ascontiguousarray(w_up[e]).astype(bf) for e in range(E)]
    wd_b = [np.ascontiguousarray(w_down[e]).astype(bf) for e in range(E)]

    in_maps = []
    scatter = []  # per core: list of (col_offset, rows, w)
    for c in range(N_CORES):
        xgt = np.zeros((D, sA + sB), dtype=bf)
        im = {"xgt": xgt}
        sc = []
        for kind, c0 in (("A", 0), ("B", sA)):
            e, rows, ws = slot_data[c].get(kind, (0, np.empty(0, np.int64),
                                                  np.empty(0, np.float32)))
            xgt[:, c0:c0 + len(rows)] = xnT_b[:, rows]
            sfx = "a" if kind == "A" else "b"
            im[f"wg_{sfx}"] = wg_b[e]
            im[f"wu_{sfx}"] = wu_b[e]
            im[f"wd_{sfx}"] = wd_b[e]
            sc.append((c0, rows, ws))
        in_maps.append(im)
        scatter.append(sc)

    # ---- device: two expert slots per core ----
    nc = _get_program(sA, sB, D, F)
    res = bass_utils.run_bass_kernel_spmd(
        nc, in_maps, core_ids=list(range(N_CORES)),
        trace=TRACE, **TRACE_KWARGS)
    LAST_RESULTS = res

    # ---- host: combine-weight scale + scatter-add ----
    out = np.zeros((T, D), dtype=np.float32)
    for c in range(N_CORES):
        yt = res.results[c]["yt"]
        for (c0, rows, ws) in scatter[c]:
            if len(rows) == 0:
                continue
            y = yt[:, c0:c0 + len(rows)].T          # [n, D] fp32
            out[rows] += ws[:, None] * y
    return out.reshape(B, S, D), router_logits


# revision 10
# speedup vs baseline: 1.2004x; 1.0198x over previous
"""MoE (top-2 of 8 experts, SwiGLU FFN) for 8 Trainium2 NeuronCores.

Strategy: expert parallelism with 2-slot load balancing. The router /
RMSNorm / top-k dispatch are O(T*D) host-side numpy; each NeuronCore runs
the SwiGLU FFN over two token slots (capacities sA >= sB), each slot bound
to one expert's weights. Every expert owns exactly two slots somewhere in
the fleet, so its token list is split across them; slot sizes are chosen
so all 16 slots cover the (data-dependent) per-expert token counts with
minimal padding. All matmuls run in bf16 with fp32 PSUM accumulation,
feature-major layout:

  slot computes   G.T = Wg.T @ Xg.T   [F, C]   (lhsT = Wg, natural layout)
                  U.T = Wu.T @ Xg.T   [F, C]
                  H.T = silu(G.T) * U.T
                  Y.T = Wd.T @ H.T    [D, C]   (lhsT = Wd, natural layout)

The host scales rows of Y by the renormalized top-2 softmax weight and
scatter-adds into the output.
"""

import numpy as np
import ml_dtypes

import concourse.bass as bass  # noqa: F401  (engine handles live on nc)
import concourse.mybir as mybir
import concourse.tile as tile
from concourse import bacc, bass_utils

EPS = 1e-6
TOP_K = 2
N_CORES = 8
P = 128

# Set by a test harness to capture profile info; default path is untouched.
TRACE = False
TRACE_KWARGS = {}
LAST_RESULTS = None

_PROG_CACHE = {}


def _ns_list(NT):
    """Split NT tokens into near-equal matmul streams (<=512 each).

    Near-equal (rather than 512+remainder) keeps every stream long enough
    (>= ~256 cycles) that the per-matmul LDWEIGHTS (~97ns) stays hidden
    behind the previous matmul's stream."""
    pieces = max(1, -(-NT // 512))
    base, rem = divmod(NT, pieces)
    out, o = [], 0
    for i in range(pieces):
        nn = base + (1 if i < rem else 0)
        out.append((o, nn))
        o += nn
    return out


def _build_program(sA, sB, D, F):
    """Two-slot SwiGLU FFN: yt[D, sA+sB] = [ffnA(xgt[:, :sA]), ffnB(xgt[:, sA:])]
    (transposed layouts), with independent weight sets per slot."""
    assert sA % 16 == 0 and sB % 16 == 0
    C = sA + sB
    KD = D // P   # contraction tiles over D (gate/up)
    KF = F // P   # contraction tiles over F (down)
    MF = F // P   # output F tiles (gate/up)
    MD = D // P   # output D tiles (down)
    bf = mybir.dt.bfloat16
    f32 = mybir.dt.float32
    AF = mybir.ActivationFunctionType

    nc = bacc.Bacc("TRN2", target_bir_lowering=False, debug=False)
    xgt = nc.dram_tensor("xgt", [D, C], bf, kind="ExternalInput").ap()
    w_in = {}
    for s in ("a", "b"):
        w_in[s] = (
            nc.dram_tensor(f"wg_{s}", [D, F], bf, kind="ExternalInput").ap(),
            nc.dram_tensor(f"wu_{s}", [D, F], bf, kind="ExternalInput").ap(),
            nc.dram_tensor(f"wd_{s}", [F, D], bf, kind="ExternalInput").ap(),
        )
    yt = nc.dram_tensor("yt", [D, C], f32, kind="ExternalOutput").ap()

    slots = [("a", 0, sA), ("b", sA, sB)]

    with tile.TileContext(nc) as tc:
        with (
            tc.tile_pool(name="xg", bufs=1) as xg_pool,
            tc.tile_pool(name="h", bufs=1) as h_pool,
            tc.tile_pool(name="wgu", bufs=6) as wgu_pool,
            tc.tile_pool(name="wdp", bufs=2) as wd_pool,
            tc.tile_pool(name="sg", bufs=4) as sg_pool,
            tc.tile_pool(name="ot", bufs=4) as o_pool,
            tc.tile_pool(name="ps", bufs=8, space="PSUM") as ps_pool,
        ):
            # whole gathered token block, K(D) on partitions: [128, KD, C].
            # Loaded slot-A-first and per ns-piece so the first matmul group
            # only waits for ~0.7MB of DMA instead of the whole block.
            xg_t = xg_pool.tile([P, KD, C], bf)
            for (_, c0, NT) in slots:
                for (n0, nn) in _ns_list(NT):
                    for k in range(KD):
                        nc.sync.dma_start(
                            xg_t[:, k, c0 + n0:c0 + n0 + nn],
                            xgt[k * P:(k + 1) * P, c0 + n0:c0 + n0 + nn])

            for (s, c0, NT) in slots:
                wg, wu, wd = w_in[s]
                ns_list = _ns_list(NT)
                h_t = h_pool.tile([P, MF, max(sA, sB)], bf, tag="h")

                # ---- gate/up projections + silu*mul -> H.T slot ----
                for mg in range(MF // 2):
                    wg_t = wgu_pool.tile([P, KD, 256], bf, tag="wgu")
                    wu_t = wgu_pool.tile([P, KD, 256], bf, tag="wgu")
                    for k in range(KD):
                        nc.sync.dma_start(
                            wg_t[:, k, :],
                            wg[k * P:(k + 1) * P, mg * 256:(mg + 1) * 256])
                        nc.sync.dma_start(
                            wu_t[:, k, :],
                            wu[k * P:(k + 1) * P, mg * 256:(mg + 1) * 256])
                    for mi in range(2):
                        m = mg * 2 + mi
                        for (n0, nn) in ns_list:
                            ps_g = ps_pool.tile([P, 512], f32, tag="ps")
                            ps_u = ps_pool.tile([P, 512], f32, tag="ps")
                            for k in range(KD):
                                nc.tensor.matmul(
                                    ps_g[:, :nn],
                                    wg_t[:, k, mi * P:(mi + 1) * P],
                                    xg_t[:, k, c0 + n0:c0 + n0 + nn],
                                    start=(k == 0), stop=(k == KD - 1))
                            for k in range(KD):
                                nc.tensor.matmul(
                                    ps_u[:, :nn],
                                    wu_t[:, k, mi * P:(mi + 1) * P],
                                    xg_t[:, k, c0 + n0:c0 + n0 + nn],
                                    start=(k == 0), stop=(k == KD - 1))
                            sg_t = sg_pool.tile([P, 512], f32, tag="sg")
                            nc.scalar.activation(sg_t[:, :nn], ps_g[:, :nn], AF.Silu)
                            nc.vector.tensor_mul(
                                h_t[:, m, n0:n0 + nn], sg_t[:, :nn], ps_u[:, :nn])

                # ---- down projection -> Y.T slot ----
                for mgd in range(MD // 2):
                    wd_t = wd_pool.tile([P, KF, 256], bf, tag="wd")
                    for k in range(KF):
                        nc.sync.dma_start(
                            wd_t[:, k, :],
                            wd[k * P:(k + 1) * P, mgd * 256:(mgd + 1) * 256])
                    for mi in range(2):
                        m = mgd * 2 + mi
                        for (n0, nn) in ns_list:
                            ps_d = ps_pool.tile([P, 512], f32, tag="ps")
                            for k in range(KF):
                                nc.tensor.matmul(
                                    ps_d[:, :nn],
                                    wd_t[:, k, mi * P:(mi + 1) * P],
                                    h_t[:, k, n0:n0 + nn],
                                    start=(k == 0), stop=(k == KF - 1))
                            o_t = o_pool.tile([P, 512], f32, tag="ot")
                            nc.vector.tensor_copy(o_t[:, :nn], ps_d[:, :nn])
                            nc.sync.dma_start(
                                yt[m * P:(m + 1) * P, c0 + n0:c0 + n0 + nn],
                                o_t[:, :nn])

    nc.compile()
    return nc


def _get_program(sA, sB, D, F):
    key = (sA, sB, D, F)
    if key not in _PROG_CACHE:
        _PROG_CACHE[key] = _build_program(sA, sB, D, F)
    return _PROG_CACHE[key]


def _assign_slots(counts):
    """Pick slot sizes (sA >= sB) and map each expert to two slots so all 16
    slots (8x sA + 8x sB) cover the per-expert token counts with minimal
    total capacity. Returns (sA, sB, pairs) where pairs[e] in
    {('A','A'), ('A','B'), ('B','B')}."""
    E = len(counts)
    G = 16  # slot-size granularity (keeps DMA rows 32B-aligned for bf16)
    max_c = int(counts.max())
    total = int(counts.sum())
    base = max(256, int(np.ceil(total / E / G)) * G)

    def greedy(sA, sB):
        order = np.argsort(-counts)
        nA, nB = E, E
        pairs = [None] * E
        for e in order:
            n = int(counts[e])
            for cap, need in ((2 * sB, ("B", "B")), (sA + sB, ("A", "B")),
                              (2 * sA, ("A", "A"))):
                a_need = need.count("A")
                b_need = need.count("B")
                if n <= cap and nA >= a_need and nB >= b_need:
                    pairs[e] = need
                    nA -= a_need
                    nB -= b_need
                    break
            else:
                return None
        return pairs

    for extra in range(0, 256 * G, G):
        s_tot = base + extra
        sA0 = int(np.ceil(s_tot / 2 / G)) * G
        for sA in range(sA0, s_tot + 1, G):
            sB = s_tot - sA
            if sB <= 0 or 2 * sA < max_c:
                continue
            pairs = greedy(sA, sB)
            if pairs is not None:
                return sA, sB, pairs
    raise RuntimeError("no feasible slot assignment")


def kernel(hidden_states, ln_weight, w_router, w_gate, w_up, w_down):
    global LAST_RESULTS
    hs = np.asarray(hidden_states, dtype=np.float32)
    ln_w = np.asarray(ln_weight, dtype=np.float32)
    w_r = np.asarray(w_router, dtype=np.float32)
    w_gate = np.asarray(w_gate)
    w_up = np.asarray(w_up)
    w_down = np.asarray(w_down)

    B, S, D = hs.shape
    T = B * S
    E, _, F = w_gate.shape
    bf = ml_dtypes.bfloat16

    # ---- host: RMSNorm + router + top-2 dispatch (O(T*D), exact fp32) ----
    x = hs.reshape(T, D)
    var = np.mean(x * x, axis=-1, keepdims=True)
    xn = x * (1.0 / np.sqrt(var + EPS)) * ln_w
    router_logits = xn @ w_r                      # [T, E]
    lm = router_logits.max(-1, keepdims=True)
    probs = np.exp(router_logits - lm)
    probs /= probs.sum(-1, keepdims=True)
    top_idx = np.argpartition(-probs, TOP_K - 1, axis=-1)[:, :TOP_K]  # [T, k]
    top_vals = np.take_along_axis(probs, top_idx, axis=-1)
    top_vals = top_vals / top_vals.sum(-1, keepdims=True)

    assert E == N_CORES, f"slot assignment assumes E == {N_CORES}, got {E}"
    flat_expert = top_idx.ravel()
    flat_token = np.repeat(np.arange(T, dtype=np.int64), TOP_K)
    flat_w = top_vals.ravel().astype(np.float32)
    counts = np.bincount(flat_expert, minlength=E)

    # ---- slot assignment: each expert -> 2 slots over the fleet ----
    sA, sB, pairs = _assign_slots(counts)
    free = {"A": list(range(N_CORES)), "B": list(range(N_CORES))}
    cap = {"A": sA, "B": sB}
    # slot_of_core[c] = {"A": (expert, rows, w), "B": ...}
    slot_data = {c: {} for c in range(N_CORES)}
    for e in range(E):
        rows = flat_token[flat_expert == e]
        ws = flat_w[flat_expert == e]
        off = 0
        for kind in pairs[e]:
            c = free[kind].pop()
            take = min(cap[kind], len(rows) - off)
            slot_data[c][kind] = (e, rows[off:off + take], ws[off:off + take])
            off += take
        assert off == len(rows), (e, off, len(rows))

    xnT_b = np.ascontiguousarray(xn.T).astype(bf)  # [D, T]
    wg_b = [np.ascontiguousarray(w_gate[e]).astype(bf) for e in range(E)]
    wu_b = [np.ascontiguousarray(w_up[e]).astype(bf) for e in range(E)]
    wd_b = [np.ascontiguousarray(w_down[e]).astype(bf) for e in range(E)]

    in_maps = []
    scatter = []  # per core: list of (col_offset, rows, w)
    for c in range(N_CORES):
        xgt = np.zeros((D, sA + sB), dtype=bf)
        im = {"xgt": xgt}
        sc = []
        for kind, c0 in (("A", 0), ("B", sA)):
            e, rows, ws = slot_data[c].get(kind, (0, np.empty(0, np.int64),
                                                  np.empty(0, np.float32)))
            xgt[:, c0:c0 + len(rows)] = xnT_b[:, rows]
            sfx = "a" if kind == "A" else "b"
            im[f"wg_{sfx}"] = wg_b[e]
            im[f"wu_{sfx}"] = wu_b[e]
            im[f"wd_{sfx}"] = wd_b[e]
            sc.append((c0, rows, ws))
        in_maps.append(im)
        scatter.append(sc)

    # ---- device: two expert slots per core ----
    nc = _get_program(sA, sB, D, F)
    res = None
    for attempt in range(3):
        try:
            res = bass_utils.run_bass_kernel_spmd(
                nc, in_maps, core_ids=list(range(N_CORES)),
                trace=TRACE, **TRACE_KWARGS)
            break
        except Exception:
            # Transient NRT device errors (e.g. NRT_EXEC_UNIT_UNRECOVERABLE
            # after a prior wedge) usually clear on retry.
            if attempt == 2:
                raise
    LAST_RESULTS = res

    # ---- host: combine-weight scale + scatter-add ----
    out = np.zeros((T, D), dtype=np.float32)
    for c in range(N_CORES):
        yt = res.results[c]["yt"]
        for (c0, rows, ws) in scatter[c]:
            if len(rows) == 0:
                continue
            y = yt[:, c0:c0 + len(rows)].T          # [n, D] fp32
            out[rows] += ws[:, None] * y
    return out.reshape(B, S, D), router_logits


# revision 14
# speedup vs baseline: 1.2247x; 1.0203x over previous
"""MoE (top-2 of 8 experts, SwiGLU FFN) for 8 Trainium2 NeuronCores.

Strategy: expert parallelism with 2-slot load balancing. The router /
RMSNorm / top-k dispatch are O(T*D) host-side numpy; each NeuronCore runs
the SwiGLU FFN over two token slots (capacities sA >= sB), each slot bound
to one expert's weights. Every expert owns exactly two slots somewhere in
the fleet, so its token list is split across them; slot sizes are chosen
so all 16 slots cover the (data-dependent) per-expert token counts with
minimal padding. All matmuls run in bf16 with fp32 PSUM accumulation,
feature-major layout:

  slot computes   G.T = Wg.T @ Xg.T   [F, C]   (lhsT = Wg, natural layout)
                  U.T = Wu.T @ Xg.T   [F, C]
                  H.T = silu(G.T) * U.T
                  Y.T = Wd.T @ H.T    [D, C]   (lhsT = Wd, natural layout)

The host scales rows of Y by the renormalized top-2 softmax weight and
scatter-adds into the output.
"""

import numpy as np
import ml_dtypes

import concourse.bass as bass  # noqa: F401  (engine handles live on nc)
import concourse.mybir as mybir
import concourse.tile as tile
from concourse import bacc, bass_utils

EPS = 1e-6
TOP_K = 2
N_CORES = 8
P = 128

# Set by a test harness to capture profile info; default path is untouched.
TRACE = False
TRACE_KWARGS = {}
LAST_RESULTS = None

_PROG_CACHE = {}


def _ns_list(NT):
    """Split NT tokens into near-equal matmul streams (<=512 each).

    Near-equal (rather than 512+remainder) keeps every stream long enough
    (>= ~256 cycles) that the per-matmul LDWEIGHTS (~97ns) stays hidden
    behind the previous matmul's stream."""
    pieces = max(1, -(-NT // 512))
    base, rem = divmod(NT, pieces)
    out, o = [], 0
    for i in range(pieces):
        nn = base + (1 if i < rem else 0)
        out.append((o, nn))
        o += nn
    return out


def _build_program(sA, sB, D, F):
    """Two-slot SwiGLU FFN: yt[D, sA+sB] = [ffnA(xgt[:, :sA]), ffnB(xgt[:, sA:])]
    (transposed layouts), with independent weight sets per slot."""
    assert sA % 16 == 0 and sB % 16 == 0
    C = sA + sB
    KD = D // P   # contraction tiles over D (gate/up)
    KF = F // P   # contraction tiles over F (down)
    MF = F // P   # output F tiles (gate/up)
    MD = D // P   # output D tiles (down)
    bf = mybir.dt.bfloat16
    f32 = mybir.dt.float32
    AF = mybir.ActivationFunctionType

    nc = bacc.Bacc("TRN2", target_bir_lowering=False, debug=False)
    xgt = nc.dram_tensor("xgt", [D, C], bf, kind="ExternalInput").ap()
    w_in = {}
    for s in ("a", "b"):
        w_in[s] = (
            nc.dram_tensor(f"wg_{s}", [D, F], bf, kind="ExternalInput").ap(),
            nc.dram_tensor(f"wu_{s}", [D, F], bf, kind="ExternalInput").ap(),
            nc.dram_tensor(f"wd_{s}", [F, D], bf, kind="ExternalInput").ap(),
        )
    yt = nc.dram_tensor("yt", [D, C], f32, kind="ExternalOutput").ap()

    slots = [("a", 0, sA), ("b", sA, sB)]

    with tile.TileContext(nc) as tc:
        with (
            tc.tile_pool(name="xg", bufs=1) as xg_pool,
            tc.tile_pool(name="h", bufs=1) as h_pool,
            tc.tile_pool(name="wgu", bufs=4) as wgu_pool,
            tc.tile_pool(name="wdp", bufs=2) as wd_pool,
            tc.tile_pool(name="sg", bufs=4) as sg_pool,
            tc.tile_pool(name="ot", bufs=4) as o_pool,
            tc.tile_pool(name="ps", bufs=8, space="PSUM") as ps_pool,
        ):
            # whole gathered token block, K(D) on partitions: [128, KD, C].
            # Loaded slot-A-first, on the SWDGE (gpsimd) queues so it runs in
            # parallel with the weight streams on the HWDGE (sync) queues —
            # the first matmul then waits ~max(first weights, slot-A tokens)
            # instead of their queue-serialized sum.
            xg_t = xg_pool.tile([P, KD, C], bf)
            for (_, c0, NT) in slots:
                for k in range(KD):
                    nc.gpsimd.dma_start(xg_t[:, k, c0:c0 + NT],
                                        xgt[k * P:(k + 1) * P, c0:c0 + NT])

            for (s, c0, NT) in slots:
                wg, wu, wd = w_in[s]
                ns_list = _ns_list(NT)
                h_t = h_pool.tile([P, MF, max(sA, sB)], bf, tag="h")

                # ---- gate/up projections + silu*mul -> H.T slot ----
                # 512-wide weight strips: DMA issue on the sync sequencer costs
                # ~606ns per 128-row transfer regardless of width, so wider
                # strips halve the issue load (it was ~94% of the PE rate at
                # 256 wide, causing periodic LDWEIGHTS stalls).
                for mg in range(MF // 4):
                    wg_t = wgu_pool.tile([P, KD, 512], bf, tag="wgu")
                    wu_t = wgu_pool.tile([P, KD, 512], bf, tag="wgu")
                    for k in range(KD):
                        nc.sync.dma_start(
                            wg_t[:, k, :],
                            wg[k * P:(k + 1) * P, mg * 512:(mg + 1) * 512])
                        nc.sync.dma_start(
                            wu_t[:, k, :],
                            wu[k * P:(k + 1) * P, mg * 512:(mg + 1) * 512])
                    for mi in range(4):
                        m = mg * 4 + mi
                        for (n0, nn) in ns_list:
                            ps_g = ps_pool.tile([P, 512], f32, tag="ps")
                            ps_u = ps_pool.tile([P, 512], f32, tag="ps")
                            for k in range(KD):
                                nc.tensor.matmul(
                                    ps_g[:, :nn],
                                    wg_t[:, k, mi * P:(mi + 1) * P],
                                    xg_t[:, k, c0 + n0:c0 + n0 + nn],
                                    start=(k == 0), stop=(k == KD - 1))
                            for k in range(KD):
                                nc.tensor.matmul(
                                    ps_u[:, :nn],
                                    wu_t[:, k, mi * P:(mi + 1) * P],
                                    xg_t[:, k, c0 + n0:c0 + n0 + nn],
                                    start=(k == 0), stop=(k == KD - 1))
                            sg_t = sg_pool.tile([P, 512], f32, tag="sg")
                            nc.scalar.activation(sg_t[:, :nn], ps_g[:, :nn], AF.Silu)
                            nc.vector.tensor_mul(
                                h_t[:, m, n0:n0 + nn], sg_t[:, :nn], ps_u[:, :nn])

                # ---- down projection -> Y.T slot ----
                # wd strips issue on the scalar engine's queues, so the down
                # phase's weight feed doesn't compete with the sync engine
                # (which is already prefetching the next phase's gate/up
                # strips and draining y-out stores).
                for mgd in range(MD // 2):
                    wd_t = wd_pool.tile([P, KF, 256], bf, tag="wd")
                    for k in range(KF):
                        nc.scalar.dma_start(
                            wd_t[:, k, :],
                            wd[k * P:(k + 1) * P, mgd * 256:(mgd + 1) * 256])
                    for mi in range(2):
                        m = mgd * 2 + mi
                        for (n0, nn) in ns_list:
                            ps_d = ps_pool.tile([P, 512], f32, tag="ps")
                            for k in range(KF):
                                nc.tensor.matmul(
                                    ps_d[:, :nn],
                                    wd_t[:, k, mi * P:(mi + 1) * P],
                                    h_t[:, k, n0:n0 + nn],
                                    start=(k == 0), stop=(k == KF - 1))
                            o_t = o_pool.tile([P, 512], f32, tag="ot")
                            nc.vector.tensor_copy(o_t[:, :nn], ps_d[:, :nn])
                            nc.sync.dma_start(
                                yt[m * P:(m + 1) * P, c0 + n0:c0 + n0 + nn],
                                o_t[:, :nn])

    nc.compile()
    return nc


def _get_program(sA, sB, D, F):
    key = (sA, sB, D, F)
    if key not in _PROG_CACHE:
        _PROG_CACHE[key] = _build_program(sA, sB, D, F)
    return _PROG_CACHE[key]


def _assign_slots(counts):
    """Pick slot sizes (sA >= sB) and map each expert to two slots so all 16
    slots (8x sA + 8x sB) cover the per-expert token counts with minimal
    total capacity. Returns (sA, sB, pairs) where pairs[e] in
    {('A','A'), ('A','B'), ('B','B')}."""
    E = len(counts)
    G = 16  # slot-size granularity (keeps DMA rows 32B-aligned for bf16)
    max_c = int(counts.max())
    total = int(counts.sum())
    base = max(256, int(np.ceil(total / E / G)) * G)

    def greedy(sA, sB):
        order = np.argsort(-counts)
        nA, nB = E, E
        pairs = [None] * E
        for e in order:
            n = int(counts[e])
            for cap, need in ((2 * sB, ("B", "B")), (sA + sB, ("A", "B")),
                              (2 * sA, ("A", "A"))):
                a_need = need.count("A")
                b_need = need.count("B")
                if n <= cap and nA >= a_need and nB >= b_need:
                    pairs[e] = need
                    nA -= a_need
                    nB -= b_need
                    break
            else:
                return None
        return pairs

    for extra in range(0, 256 * G, G):
        s_tot = base + extra
        sA0 = int(np.ceil(s_tot / 2 / G)) * G
        for sA in range(sA0, s_tot + 1, G):
            sB = s_tot - sA
            if sB <= 0 or 2 * sA < max_c:
                continue
            pairs = greedy(sA, sB)
            if pairs is not None:
                return sA, sB, pairs
    raise RuntimeError("no feasible slot assignment")


def kernel(hidden_states, ln_weight, w_router, w_gate, w_up, w_down):
    global LAST_RESULTS
    hs = np.asarray(hidden_states, dtype=np.float32)
    ln_w = np.asarray(ln_weight, dtype=np.float32)
    w_r = np.asarray(w_router, dtype=np.float32)
    w_gate = np.asarray(w_gate)
    w_up = np.asarray(w_up)
    w_down = np.asarray(w_down)

    B, S, D = hs.shape
    T = B * S
    E, _, F = w_gate.shape
    bf = ml_dtypes.bfloat16

    # ---- host: RMSNorm + router + top-2 dispatch (O(T*D), exact fp32) ----
    x = hs.reshape(T, D)
    var = np.mean(x * x, axis=-1, keepdims=True)
    xn = x * (1.0 / np.sqrt(var + EPS)) * ln_w
    router_logits = xn @ w_r                      # [T, E]
    lm = router_logits.max(-1, keepdims=True)
    probs = np.exp(router_logits - lm)
    probs /= probs.sum(-1, keepdims=True)
    top_idx = np.argpartition(-probs, TOP_K - 1, axis=-1)[:, :TOP_K]  # [T, k]
    top_vals = np.take_along_axis(probs, top_idx, axis=-1)
    top_vals = top_vals / top_vals.sum(-1, keepdims=True)

    assert E == N_CORES, f"slot assignment assumes E == {N_CORES}, got {E}"
    flat_expert = top_idx.ravel()
    flat_token = np.repeat(np.arange(T, dtype=np.int64), TOP_K)
    flat_w = top_vals.ravel().astype(np.float32)
    counts = np.bincount(flat_expert, minlength=E)

    # ---- slot assignment: each expert -> 2 slots over the fleet ----
    sA, sB, pairs = _assign_slots(counts)
    free = {"A": list(range(N_CORES)), "B": list(range(N_CORES))}
    cap = {"A": sA, "B": sB}
    # slot_of_core[c] = {"A": (expert, rows, w), "B": ...}
    slot_data = {c: {} for c in range(N_CORES)}
    for e in range(E):
        rows = flat_token[flat_expert == e]
        ws = flat_w[flat_expert == e]
        off = 0
        for kind in pairs[e]:
            c = free[kind].pop()
            take = min(cap[kind], len(rows) - off)
            slot_data[c][kind] = (e, rows[off:off + take], ws[off:off + take])
            off += take
        assert off == len(rows), (e, off, len(rows))

    xnT_b = np.ascontiguousarray(xn.T).astype(bf)  # [D, T]
    wg_b = [np.ascontiguousarray(w_gate[e]).astype(bf) for e in range(E)]
    wu_b = [np.ascontiguousarray(w_up[e]).astype(bf) for e in range(E)]
    wd_b = [np.ascontiguousarray(w_down[e]).astype(bf) for e in range(E)]

    in_maps = []
    scatter = []  # per core: list of (col_offset, rows, w)
    for c in range(N_CORES):
        xgt = np.zeros((D, sA + sB), dtype=bf)
        im = {"xgt": xgt}
        sc = []
        for kind, c0 in (("A", 0), ("B", sA)):
            e, rows, ws = slot_data[c].get(kind, (0, np.empty(0, np.int64),
                                                  np.empty(0, np.float32)))
            xgt[:, c0:c0 + len(rows)] = xnT_b[:, rows]
            sfx = "a" if kind == "A" else "b"
            im[f"wg_{sfx}"] = wg_b[e]
            im[f"wu_{sfx}"] = wu_b[e]
            im[f"wd_{sfx}"] = wd_b[e]
            sc.append((c0, rows, ws))
        in_maps.append(im)
        scatter.append(sc)

    # ---- device: two expert slots per core ----
    nc = _get_program(sA, sB, D, F)
    res = None
    for attempt in range(3):
        try:
            res = bass_utils.run_bass_kernel_spmd(
                nc, in_maps, core_ids=list(range(N_CORES)),
                trace=TRACE, **TRACE_KWARGS)
            break
        except Exception:
            # Transient NRT device errors (e.g. NRT_EXEC_UNIT_UNRECOVERABLE
            # after a prior wedge) usually clear on retry.
            if attempt == 2:
                raise
    LAST_RESULTS = res

    # ---- host: combine-weight scale + scatter-add ----
    out = np.zeros((T, D), dtype=np.float32)
    for c in range(N_CORES):
        yt = res.results[c]["yt"]
        for (c0, rows, ws) in scatter[c]:
            if len(rows) == 0:
                continue
            y = yt[:, c0:c0 + len(rows)].T          # [n, D] fp32
            out[rows] += ws[:, None] * y
    return out.reshape(B, S, D), router_logits


# revision 15
# speedup vs baseline: 1.2288x; 1.0033x over previous
"""MoE (top-2 of 8 experts, SwiGLU FFN) for 8 Trainium2 NeuronCores.

Strategy: expert parallelism with 2-slot load balancing. The router /
RMSNorm / top-k dispatch are O(T*D) host-side numpy; each NeuronCore runs
the SwiGLU FFN over two token slots (capacities sA >= sB), each slot bound
to one expert's weights. Every expert owns exactly two slots somewhere in
the fleet, so its token list is split across them; slot sizes are chosen
so all 16 slots cover the (data-dependent) per-expert token counts with
minimal padding. All matmuls run in bf16 with fp32 PSUM accumulation,
feature-major layout:

  slot computes   G.T = Wg.T @ Xg.T   [F, C]   (lhsT = Wg, natural layout)
                  U.T = Wu.T @ Xg.T   [F, C]
                  H.T = silu(G.T) * U.T
                  Y.T = Wd.T @ H.T    [D, C]   (lhsT = Wd, natural layout)

The host scales rows of Y by the renormalized top-2 softmax weight and
scatter-adds into the output.
"""

import numpy as np
import ml_dtypes

import concourse.bass as bass  # noqa: F401  (engine handles live on nc)
import concourse.mybir as mybir
import concourse.tile as tile
from concourse import bacc, bass_utils

EPS = 1e-6
TOP_K = 2
N_CORES = 8
P = 128

# Set by a test harness to capture profile info; default path is untouched.
TRACE = False
TRACE_KWARGS = {}
LAST_RESULTS = None

_PROG_CACHE = {}


def _ns_list(NT):
    """Split NT tokens into near-equal matmul streams (<=512 each).

    Near-equal (rather than 512+remainder) keeps every stream long enough
    (>= ~256 cycles) that the per-matmul LDWEIGHTS (~97ns) stays hidden
    behind the previous matmul's stream."""
    pieces = max(1, -(-NT // 512))
    base, rem = divmod(NT, pieces)
    out, o = [], 0
    for i in range(pieces):
        nn = base + (1 if i < rem else 0)
        out.append((o, nn))
        o += nn
    return out


def _build_program(sA, sB, D, F):
    """Two-slot SwiGLU FFN: yt[D, sA+sB] = [ffnA(xgt[:, :sA]), ffnB(xgt[:, sA:])]
    (transposed layouts), with independent weight sets per slot."""
    assert sA % 16 == 0 and sB % 16 == 0
    C = sA + sB
    KD = D // P   # contraction tiles over D (gate/up)
    KF = F // P   # contraction tiles over F (down)
    MF = F // P   # output F tiles (gate/up)
    MD = D // P   # output D tiles (down)
    bf = mybir.dt.bfloat16
    f32 = mybir.dt.float32
    AF = mybir.ActivationFunctionType

    nc = bacc.Bacc("TRN2", target_bir_lowering=False, debug=False)
    xgt = nc.dram_tensor("xgt", [D, C], bf, kind="ExternalInput").ap()
    w_in = {}
    for s in ("a", "b"):
        w_in[s] = (
            nc.dram_tensor(f"wg_{s}", [D, F], bf, kind="ExternalInput").ap(),
            nc.dram_tensor(f"wu_{s}", [D, F], bf, kind="ExternalInput").ap(),
            nc.dram_tensor(f"wd_{s}", [F, D], bf, kind="ExternalInput").ap(),
        )
    yt = nc.dram_tensor("yt", [D, C], f32, kind="ExternalOutput").ap()

    slots = [("a", 0, sA), ("b", sA, sB)]

    with tile.TileContext(nc) as tc:
        with (
            tc.tile_pool(name="xg", bufs=1) as xg_pool,
            tc.tile_pool(name="h", bufs=1) as h_pool,
            tc.tile_pool(name="wgu", bufs=4) as wgu_pool,
            tc.tile_pool(name="wdp", bufs=2) as wd_pool,
            tc.tile_pool(name="sg", bufs=4) as sg_pool,
            tc.tile_pool(name="ot", bufs=4) as o_pool,
            tc.tile_pool(name="ps", bufs=8, space="PSUM") as ps_pool,
        ):
            # whole gathered token block, K(D) on partitions: [128, KD, C].
            # Slot A rides the scalar engine's HWDGE queues (scalar is idle
            # until the first down phase) so the startup token load runs at
            # HW-queue rate, in parallel with the gate weights on sync's
            # queues; slot B trickles in on the gpsimd SWDGE queues — it
            # isn't needed until ~halfway through the kernel.
            xg_t = xg_pool.tile([P, KD, C], bf)
            for (s, c0, NT) in slots:
                dma_eng = nc.scalar if s == "a" else nc.gpsimd
                for k in range(KD):
                    dma_eng.dma_start(xg_t[:, k, c0:c0 + NT],
                                      xgt[k * P:(k + 1) * P, c0:c0 + NT])

            for (s, c0, NT) in slots:
                wg, wu, wd = w_in[s]
                ns_list = _ns_list(NT)
                h_t = h_pool.tile([P, MF, max(sA, sB)], bf, tag="h")

                # ---- gate/up projections + silu*mul -> H.T slot ----
                # 512-wide weight strips: DMA issue on the sync sequencer costs
                # ~606ns per 128-row transfer regardless of width, so wider
                # strips halve the issue load (it was ~94% of the PE rate at
                # 256 wide, causing periodic LDWEIGHTS stalls).
                for mg in range(MF // 4):
                    wg_t = wgu_pool.tile([P, KD, 512], bf, tag="wgu")
                    wu_t = wgu_pool.tile([P, KD, 512], bf, tag="wgu")
                    for k in range(KD):
                        nc.sync.dma_start(
                            wg_t[:, k, :],
                            wg[k * P:(k + 1) * P, mg * 512:(mg + 1) * 512])
                        nc.sync.dma_start(
                            wu_t[:, k, :],
                            wu[k * P:(k + 1) * P, mg * 512:(mg + 1) * 512])
                    for mi in range(4):
                        m = mg * 4 + mi
                        for (n0, nn) in ns_list:
                            ps_g = ps_pool.tile([P, 512], f32, tag="ps")
                            ps_u = ps_pool.tile([P, 512], f32, tag="ps")
                            for k in range(KD):
                                nc.tensor.matmul(
                                    ps_g[:, :nn],
                                    wg_t[:, k, mi * P:(mi + 1) * P],
                                    xg_t[:, k, c0 + n0:c0 + n0 + nn],
                                    start=(k == 0), stop=(k == KD - 1))
                            for k in range(KD):
                                nc.tensor.matmul(
                                    ps_u[:, :nn],
                                    wu_t[:, k, mi * P:(mi + 1) * P],
                                    xg_t[:, k, c0 + n0:c0 + n0 + nn],
                                    start=(k == 0), stop=(k == KD - 1))
                            sg_t = sg_pool.tile([P, 512], f32, tag="sg")
                            nc.scalar.activation(sg_t[:, :nn], ps_g[:, :nn], AF.Silu)
                            nc.vector.tensor_mul(
                                h_t[:, m, n0:n0 + nn], sg_t[:, :nn], ps_u[:, :nn])

                # ---- down projection -> Y.T slot ----
                # wd strips issue on the scalar engine's queues, so the down
                # phase's weight feed doesn't compete with the sync engine
                # (which is already prefetching the next phase's gate/up
                # strips and draining y-out stores).
                for mgd in range(MD // 2):
                    wd_t = wd_pool.tile([P, KF, 256], bf, tag="wd")
                    for k in range(KF):
                        nc.scalar.dma_start(
                            wd_t[:, k, :],
                            wd[k * P:(k + 1) * P, mgd * 256:(mgd + 1) * 256])
                    for mi in range(2):
                        m = mgd * 2 + mi
                        for (n0, nn) in ns_list:
                            ps_d = ps_pool.tile([P, 512], f32, tag="ps")
                            for k in range(KF):
                                nc.tensor.matmul(
                                    ps_d[:, :nn],
                                    wd_t[:, k, mi * P:(mi + 1) * P],
                                    h_t[:, k, n0:n0 + nn],
                                    start=(k == 0), stop=(k == KF - 1))
                            o_t = o_pool.tile([P, 512], f32, tag="ot")
                            nc.vector.tensor_copy(o_t[:, :nn], ps_d[:, :nn])
                            nc.sync.dma_start(
                                yt[m * P:(m + 1) * P, c0 + n0:c0 + n0 + nn],
                                o_t[:, :nn])

    nc.compile()
    return nc


def _get_program(sA, sB, D, F):
    key = (sA, sB, D, F)
    if key not in _PROG_CACHE:
        _PROG_CACHE[key] = _build_program(sA, sB, D, F)
    return _PROG_CACHE[key]


def _assign_slots(counts):
    """Pick slot sizes (sA >= sB) and map each expert to two slots so all 16
    slots (8x sA + 8x sB) cover the per-expert token counts with minimal
    total capacity. Returns (sA, sB, pairs) where pairs[e] in
    {('A','A'), ('A','B'), ('B','B')}."""
    E = len(counts)
    G = 16  # slot-size granularity (keeps DMA rows 32B-aligned for bf16)
    max_c = int(counts.max())
    total = int(counts.sum())
    base = max(256, int(np.ceil(total / E / G)) * G)

    def greedy(sA, sB):
        order = np.argsort(-counts)
        nA, nB = E, E
        pairs = [None] * E
        for e in order:
            n = int(counts[e])
            for cap, need in ((2 * sB, ("B", "B")), (sA + sB, ("A", "B")),
                              (2 * sA, ("A", "A"))):
                a_need = need.count("A")
                b_need = need.count("B")
                if n <= cap and nA >= a_need and nB >= b_need:
                    pairs[e] = need
                    nA -= a_need
                    nB -= b_need
                    break
            else:
                return None
        return pairs

    for extra in range(0, 256 * G, G):
        s_tot = base + extra
        sA0 = int(np.ceil(s_tot / 2 / G)) * G
        for sA in range(sA0, s_tot + 1, G):
            sB = s_tot - sA
            if sB <= 0 or 2 * sA < max_c:
                continue
            pairs = greedy(sA, sB)
            if pairs is not None:
                return sA, sB, pairs
    raise RuntimeError("no feasible slot assignment")


def kernel(hidden_states, ln_weight, w_router, w_gate, w_up, w_down):
    global LAST_RESULTS
    hs = np.asarray(hidden_states, dtype=np.float32)
    ln_w = np.asarray(ln_weight, dtype=np.float32)
    w_r = np.asarray(w_router, dtype=np.float32)
    w_gate = np.asarray(w_gate)
    w_up = np.asarray(w_up)
    w_down = np.asarray(w_down)

    B, S, D = hs.shape
    T = B * S
    E, _, F = w_gate.shape
    bf = ml_dtypes.bfloat16

    # ---- host: RMSNorm + router + top-2 dispatch (O(T*D), exact fp32) ----
    x = hs.reshape(T, D)
    var = np.mean(x * x, axis=-1, keepdims=True)
    xn = x * (1.0 / np.sqrt(var + EPS)) * ln_w
    router_logits = xn @ w_r                      # [T, E]
    lm = router_logits.max(-1, keepdims=True)
    probs = np.exp(router_logits - lm)
    probs /= probs.sum(-1, keepdims=True)
    top_idx = np.argpartition(-probs, TOP_K - 1, axis=-1)[:, :TOP_K]  # [T, k]
    top_vals = np.take_along_axis(probs, top_idx, axis=-1)
    top_vals = top_vals / top_vals.sum(-1, keepdims=True)

    assert E == N_CORES, f"slot assignment assumes E == {N_CORES}, got {E}"
    flat_expert = top_idx.ravel()
    flat_token = np.repeat(np.arange(T, dtype=np.int64), TOP_K)
    flat_w = top_vals.ravel().astype(np.float32)
    counts = np.bincount(flat_expert, minlength=E)

    # ---- slot assignment: each expert -> 2 slots over the fleet ----
    sA, sB, pairs = _assign_slots(counts)
    free = {"A": list(range(N_CORES)), "B": list(range(N_CORES))}
    cap = {"A": sA, "B": sB}
    # slot_of_core[c] = {"A": (expert, rows, w), "B": ...}
    slot_data = {c: {} for c in range(N_CORES)}
    for e in range(E):
        rows = flat_token[flat_expert == e]
        ws = flat_w[flat_expert == e]
        off = 0
        for kind in pairs[e]:
            c = free[kind].pop()
            take = min(cap[kind], len(rows) - off)
            slot_data[c][kind] = (e, rows[off:off + take], ws[off:off + take])
            off += take
        assert off == len(rows), (e, off, len(rows))

    xnT_b = np.ascontiguousarray(xn.T).astype(bf)  # [D, T]
    wg_b = [np.ascontiguousarray(w_gate[e]).astype(bf) for e in range(E)]
    wu_b = [np.ascontiguousarray(w_up[e]).astype(bf) for e in range(E)]
    wd_b = [np.ascontiguousarray(w_down[e]).astype(bf) for e in range(E)]

    in_maps = []
    scatter = []  # per core: list of (col_offset, rows, w)
    for c in range(N_CORES):
        xgt = np.zeros((D, sA + sB), dtype=bf)
        im = {"xgt": xgt}
        sc = []
        for kind, c0 in (("A", 0), ("B", sA)):
            e, rows, ws = slot_data[c].get(kind, (0, np.empty(0, np.int64),
                                                  np.empty(0, np.float32)))
            xgt[:, c0:c0 + len(rows)] = xnT_b[:, rows]
            sfx = "a" if kind == "A" else "b"
            im[f"wg_{sfx}"] = wg_b[e]
            im[f"wu_{sfx}"] = wu_b[e]
            im[f"wd_{sfx}"] = wd_b[e]
            sc.append((c0, rows, ws))
        in_maps.append(im)
        scatter.append(sc)

    # ---- device: two expert slots per core ----
    nc = _get_program(sA, sB, D, F)
    res = None
    for attempt in range(3):
        try:
            res = bass_utils.run_bass_kernel_spmd(
                nc, in_maps, core_ids=list(range(N_CORES)),
                trace=TRACE, **TRACE_KWARGS)
            break
        except Exception:
            # Transient NRT device errors (e.g. NRT_EXEC_UNIT_UNRECOVERABLE
            # after a prior wedge) usually clear on retry.
            if attempt == 2:
                raise
    LAST_RESULTS = res

    # ---- host: combine-weight scale + scatter-add ----
    out = np.zeros((T, D), dtype=np.float32)
    for c in range(N_CORES):
        yt = res.results[c]["yt"]
        for (c0, rows, ws) in scatter[c]:
            if len(rows) == 0:
                continue
            y = yt[:, c0:c0 + len(rows)].T          # [n, D] fp32
            out[rows] += ws[:, None] * y
    return out.reshape(B, S, D), router_logits
